# revision 2
# baseline (speedup 1.0000x reference)
"""VQ Euclidean-codebook kernel for Trainium2 (8 NeuronCores, data-parallel).

Math: quantize[n] = embed[argmin_k ||x[n]-embed[k]||^2]
    = embed[argmax_k (x[n].e_k - 0.5||e_k||^2)]

v4 design (vs baseline: DVE was the wall with a full-K 1x argmax scan,
~4.4us/tile):
  - PE (fp16 two-pass, exact to ~2e-6): P1 = [x_h; r_x].[e_h; e_h] (C=128),
    P2 = [x_h; 1; 1].[r_e; e2_hi; e2_lo] (C=66) accumulated in PSUM. The
    -0.5||e||^2 term is folded into P2 via two fp16 aug rows (hi+lo split,
    exact to ~7e-6); only r_x.r_e (~2e-6) is dropped. 16 matmuls W=512 per
    tile = 8192 PE cycles = 3413ns -> the intended bottleneck.
  - Drain: ACT copies PSUM chunks 0-2 (3x1024) to SBUF, DVE copies chunk 3.
  - DVE: ONE fused custom scan op over the two K-halves:
    v = max(Src0=sc[:, :2048], Src1=sc[:, 2048:]), running-max scan,
    qual = (v == runmax), out = select(qual, Idx, -1), accum MAX ->
    pair-index kp in [0,2048). Halves the scan length vs baseline.
  - GPSIMD: resolves which half won: gathers sc[p, kp] and sc[p, kp+2048]
    via indirect_copy, tag = (B > A), k = kp + 2048*tag; then the usual
    indirect-DMA gather of embed[k] and HWDGE store.
"""

import numpy as np

import concourse.bass as bass
import concourse.bacc as bacc
import concourse.mybir as mybir
from concourse.tile import TileContext
from concourse.bass_utils import run_bass_kernel_spmd

from concourse import dve_ops
from concourse.dve_spec import (
    Spec, Src0, Src1, AluOp, Idx, scan, select, eq, lower, maxx, C0, C1, C2, Bin,
)
from concourse.dve_uop import DveOpSpec

P = 128          # partitions / rows per tile
N_FULL = 131072  # total rows
N_CORES = 8
N_LOC = N_FULL // N_CORES  # 16384
K = 4096         # codebook size
KH = K // 2      # 2048: pair-scan half
D = 64           # feature dim
NT = N_LOC // P  # 128 tiles per core
F32 = mybir.dt.float32
F16 = mybir.dt.float16
U16 = mybir.dt.uint16
I32 = mybir.dt.int32

_OP_NAME = "PAIRMAX_ARGMAX_ANT"
_TAG_NAME = "TAGSEL_ANT"


def _pairmax_argmax_reference(in0, in1, c0, c1, c2):
    a = np.asarray(in0, np.float32)
    b = np.asarray(in1, np.float32)
    v = np.maximum(a, b)
    v2 = v.reshape(v.shape[0], -1)
    r = np.maximum.accumulate(v2, axis=1)
    qual = v2 == r
    idxs = np.arange(v2.shape[1], dtype=np.float32)[None, :]
    body = np.where(qual, idxs, np.float32(c2)).astype(np.float32)
    acc = body.max(axis=1, keepdims=True)
    return body.reshape(in0.shape), acc


def _tagsel_reference(in0, in1, c0, c1, c2):
    a = np.asarray(in0, np.float32)
    b = np.asarray(in1, np.float32)
    out = (np.asarray(c0, np.float32)
           + (b > a).astype(np.float32) * np.float32(c1)).astype(np.float32)
    return out, None


def _make_maskpm():
    # [128, 32] f32: -1 at col (p%16), +1 at col 16+(p%16): masked reduce_sum
    # of the wrapped indirect_copy output gives vb - va per partition.
    m = np.zeros((128, 32), np.float32)
    for p in range(128):
        m[p, p % 16] = -1.0
        m[p, 16 + p % 16] = 1.0
    return m


def _register(name, spec, rd1_en=True):
    for op in dve_ops.OPS:
        if op.name == name:
            return op
    row = dve_ops._CUSTOM_DVE_ROW_BASE + len(dve_ops.OPS)
    dve_ops._SUB_OPCODE_FOR_NAME[name] = row
    uops = lower(spec, ver="v3")
    sha = DveOpSpec(name=name, opcode=row, uops=uops, rd1_en=rd1_en).sha("v3")
    op = dve_ops.DveOp(name, spec, subdim=False, uops_sha={"v3": sha})
    dve_ops.OPS.append(op)
    dve_ops.CUSTOM_DVE_SPECS[name] = spec
    return op


def register_ops():
    v = maxx(Src0, Src1)
    body = select(eq(v, scan(AluOp.MAX, v)), Idx, C2)
    pair_op = _register(_OP_NAME, Spec(
        body=body, accum=AluOp.MAX, reference=_pairmax_argmax_reference))
    tag_body = Bin(AluOp.ADD, C0,
                   Bin(AluOp.MULTIPLY, Bin(AluOp.IS_GT, Src1, Src0), C1))
    tag_op = _register(_TAG_NAME, Spec(
        body=tag_body, reference=_tagsel_reference))
    return pair_op, tag_op


def build(r_iters: int = 1, stage: int = 2):
    """stage: 0 = PE+drains only, 1 = +pair-scan+converts+gather32, 2 = full."""
    pair_op, tag_op = register_ops()
    nc = bacc.Bacc(num_devices=N_CORES)
    xs1_in = nc.dram_tensor("xs1", [128, N_LOC], F16, kind="ExternalInput")
    xs2_in = nc.dram_tensor("xs2", [66, N_LOC], F16, kind="ExternalInput")
    rhs1_in = nc.dram_tensor("rhs1", [128, K], F16, kind="ExternalInput")
    rhs2_in = nc.dram_tensor("rhs2", [66, K], F16, kind="ExternalInput")
    maskpm_in = nc.dram_tensor("maskpm", [P, 32], F32, kind="ExternalInput")
    emb_in = nc.dram_tensor("embed", [K, D], F32, kind="ExternalInput")
    q_out = nc.dram_tensor("q", [N_LOC, D], F32, kind="ExternalOutput")

    # psum chunks: (width, drain-engine); emitted as P1(c),P1(c+1),P2(c),P2(c+1)
    # pairs so the stationary only alternates twice per tile, while chunk
    # completion times stay early enough for drains to pipeline under T=3413.
    CHUNKS = [(1024, "act"), (1024, "act"), (512, "dve"), (1536, "act")]

    with TileContext(nc) as tc:
        with (
            tc.tile_pool(name="const", bufs=1) as cpool,
            tc.tile_pool(name="score", bufs=4) as spool,
            tc.tile_pool(name="junk", bufs=1) as jpool,
            tc.tile_pool(name="idx", bufs=4) as ipool,
            tc.tile_pool(name="gather", bufs=4) as gpool,
            tc.tile_pool(name="ps", bufs=1, space="PSUM") as pspool,
        ):
            xs1 = cpool.tile([128, N_LOC], F16)
            nc.sync.dma_start(out=xs1[:, :], in_=xs1_in[:, :])
            xs2 = cpool.tile([66, N_LOC], F16)
            nc.sync.dma_start(out=xs2[:, :], in_=xs2_in[:, :])
            rhs1 = cpool.tile([128, K], F16)
            nc.sync.dma_start(out=rhs1[:, :], in_=rhs1_in[:, :])
            rhs2 = cpool.tile([66, K], F16)
            nc.sync.dma_start(out=rhs2[:, :], in_=rhs2_in[:, :])
            maskpm = cpool.tile([P, 32], F32)
            nc.sync.dma_start(out=maskpm[:, :], in_=maskpm_in[:, :])
            zero1 = cpool.tile([P, 1], F32)
            nc.vector.memset(zero1[:, :], 0.0)

            def tile_head(t):
                """PE + drains + pair-scan + index converts + GPS gather32.
                Returns handles the deferred tail needs."""
                nsl = slice(t * P, (t + 1) * P)
                sc = spool.tile([P, K], F32, tag="sc")
                offs, o = [], 0
                for cw, _ in CHUNKS:
                    offs.append(o)
                    o += cw
                tiles = []
                for ci, (cw, _) in enumerate(CHUNKS):
                    pst = pspool.tile([P, cw], F32, tag=f"ps{ci}", name=f"pst{ci}")
                    tiles.append(pst)

                def mm(pass_, ci):
                    lhs, rhs = (xs1, rhs1) if pass_ == 1 else (xs2, rhs2)
                    cw = CHUNKS[ci][0]
                    for q in range(cw // 512):
                        csl = slice(offs[ci] + q * 512, offs[ci] + (q + 1) * 512)
                        psl = slice(q * 512, (q + 1) * 512)
                        nc.tensor.matmul(out=tiles[ci][:, psl], lhsT=lhs[:, nsl],
                                         rhs=rhs[:, csl],
                                         start=(pass_ == 1), stop=(pass_ == 2))

                def drain(ci):
                    cw, eng = CHUNKS[ci]
                    osl = slice(offs[ci], offs[ci] + cw)
                    if eng == "act":
                        nc.scalar.copy(out=sc[:, osl], in_=tiles[ci][:, :])
                    else:
                        nc.vector.tensor_copy(out=sc[:, osl], in_=tiles[ci][:, :])

                for c0 in (0, 2):
                    mm(1, c0); mm(1, c0 + 1); mm(2, c0); drain(c0)
                    mm(2, c0 + 1); drain(c0 + 1)
                if stage == 0:
                    return None, None

                junk = jpool.tile([P, KH], F32)
                kpf = ipool.tile([P, 1], F32, tag="kpf")
                nc.vector._custom_dve(
                    pair_op, out=junk[:, :], in0=sc[:, 0:KH], in1=sc[:, KH:K],
                    imm2=-1.0, accum_out=kpf[:, :],
                )
                # packed u16 index pair [kp, kp+KH] for the wrapped gather
                iidx = ipool.tile([P, 2], U16, tag="iidx")
                nc.vector.tensor_copy(out=iidx[:, 0:1], in_=kpf[:, :])
                nc.vector.tensor_scalar_add(iidx[:, 1:2], iidx[:, 0:1], KH)
                # GPS: out32[p, j] = sc[p, idxlist_g[j]]; va at col p%16,
                # vb at col 16+p%16 (wrapped 16-partition semantics)
                g32 = None
                if stage != 4:
                    g32 = ipool.tile([P, 32], F32, tag="g32")
                    nc.gpsimd.indirect_copy(g32[:, :], sc[:, :], iidx[:, :], True)
                return kpf, g32

            def tile_tail(t, kpf, g32):
                if stage < 2 or kpf is None:
                    return
                if stage == 4:
                    # timing probe: gather embed[kp] directly (wrong result)
                    ki = ipool.tile([P, 1], I32, tag="ki")
                    nc.vector.tensor_copy(out=ki[:, :], in_=kpf[:, :])
                else:
                    # d = vb - va via two-hot masked sum; kf = kp + KH*(d>0)
                    md = ipool.tile([P, 32], F32, tag="md")
                    nc.vector.tensor_mul(md[:, :], g32[:, :], maskpm[:, :])
                    dd = ipool.tile([P, 1], F32, tag="dd")
                    nc.vector.reduce_sum(out=dd[:, :], in_=md[:, :],
                                         axis=mybir.AxisListType.X)
                    kf = ipool.tile([P, 1], F32, tag="kf")
                    nc.vector._custom_dve(
                        tag_op, out=kf[:, :], in0=zero1[:, :], in1=dd[:, :],
                        s0=kpf[:, :1], s1=float(KH),
                    )
                    ki = ipool.tile([P, 1], I32, tag="ki")
                    nc.vector.tensor_copy(out=ki[:, :], in_=kf[:, :])
                if stage == 3:
                    return
                g = gpool.tile([P, D], F32)
                nc.gpsimd.indirect_dma_start(
                    out=g[:, :], out_offset=None, in_=emb_in[:, :],
                    in_offset=bass.IndirectOffsetOnAxis(ap=ki[:, :1], axis=0),
                )
                nc.sync.dma_start(out=q_out[t * P:(t + 1) * P, :], in_=g[:, :])

            def tile_loop():
                prev = None
                for t in range(NT):
                    cur = tile_head(t)
                    if prev is not None:
                        tile_tail(t - 1, *prev)
                    prev = cur
                tile_tail(NT - 1, *prev)

            if r_iters == 1:
                tile_loop()
            else:
                with tc.For_i(0, r_iters, 1):
                    tile_loop()

    nc.compile()
    return nc


def make_in_maps(x: np.ndarray, embed: np.ndarray):
    x = np.ascontiguousarray(x, dtype=np.float32)
    embed = np.ascontiguousarray(embed, dtype=np.float32)

    e16 = embed.astype(np.float16)
    re_ = (embed - e16.astype(np.float32)).astype(np.float16)
    e2 = (-0.5 * (embed.astype(np.float64) ** 2).sum(1)).astype(np.float32)
    e2hi = e2.astype(np.float16)
    e2lo = (e2 - e2hi.astype(np.float32)).astype(np.float16)
    # rhs1 [128, K] = [e_h; e_h];  rhs2 [66, K] = [r_e; e2hi; e2lo]
    rhs1 = np.concatenate([e16.T, e16.T], axis=0)
    rhs2 = np.concatenate(
        [re_.T, e2hi[None, :], e2lo[None, :]], axis=0)

    maps = []
    for c in range(N_CORES):
        xs = x[c * N_LOC:(c + 1) * N_LOC]          # [N_LOC, 64]
        xh = xs.astype(np.float16)
        rx = (xs - xh.astype(np.float32)).astype(np.float16)
        ones = np.ones((2, N_LOC), np.float16)
        xs1 = np.concatenate([xh.T, rx.T], axis=0)  # [128, N_LOC]
        xs2 = np.concatenate([xh.T, ones], axis=0)  # [66, N_LOC]
        maps.append({
            "xs1": np.ascontiguousarray(xs1),
            "xs2": np.ascontiguousarray(xs2),
            "rhs1": np.ascontiguousarray(rhs1.astype(np.float16)),
            "rhs2": np.ascontiguousarray(rhs2.astype(np.float16)),
            "maskpm": _make_maskpm(),
            "embed": embed,
        })
    return maps


_CACHED_NC = None


def kernel(x: np.ndarray, embed: np.ndarray) -> np.ndarray:
    global _CACHED_NC
    assert x.shape == (N_FULL, D) and embed.shape == (K, D)
    if _CACHED_NC is None:
        _CACHED_NC = build()
    res = run_bass_kernel_spmd(
        _CACHED_NC, make_in_maps(x, embed), core_ids=list(range(N_CORES))
    )
    return np.concatenate([r["q"] for r in res.results], axis=0)
